# revision 1
# baseline (speedup 1.0000x reference)
"""CPGNN (compatibility-guided GNN) kernel for 8 Trainium2 NeuronCores.

Reference computation (N=10000, F=512, HID=256, C=16, 4 post iterations):
    h      = relu(normed_adj @ (features @ W1) + b1)
    logits = normed_adj @ (h @ W2) + b2
    E_hat  = softmax(logits) - 1/C
    B_hat  = E_hat;  4x: B_hat = E_hat + raw_adj @ (B_hat @ H)
    out    = B_hat + 1/C

Sharding: rows of both adjacency matrices are sharded over the 8 cores
(1280 rows per core, tail core padded).  The adjacency shards are
uploaded TRANSPOSED (K-major, [10240, 1280] bf16) so every on-device
matmul can use natural layouts.  The small per-core [rows, C] matrices
are all-gathered between phases via device collectives.  All big
matmuls run in bf16 with fp32 PSUM accumulation (verified: end-to-end
relative error ~3.7e-3, at the fp32 reordering noise floor of this
amplifying iteration).
"""

import os

import numpy as np
import ml_dtypes

RANKS = 8
P = 128
NREAL = 10000
NK = 10240            # padded global row count (80 k-tiles)
ML = 1280             # local rows per core (10 m-tiles)
KT = NK // P          # 80
MT = ML // P          # 10
F = 512
FT = F // P           # 4
HID = 256
C = 16
NPOST = 4
NRES = 50             # raw-adj k-tiles kept resident in SBUF after 1st pass
NRESN = 32            # normed-adj k-tiles cached in SBUF between ph2 and ph4
NCH = 3               # free-dim chunks of ML: 512/512/256
CHUNKS = [(0, 512), (512, 1024), (1024, 1280)]

PHASES = int(os.environ.get("CPGNN_PHASES", "5"))

_CACHE = {}


def _mix_order(n_cache, n_total):
    """Interleave cached (0..n_cache-1) and streamed (n_cache..) k indices so
    DMA of streamed tiles overlaps PE work on cached tiles evenly."""
    cached = list(range(n_cache))
    streamed = list(range(n_cache, n_total))
    order = []
    ic = si = 0
    for i in range(n_total):
        want_stream = streamed and (si + 1) / len(streamed) <= (i + 1) / n_total
        if si < len(streamed) and (ic >= len(cached) or want_stream):
            order.append(streamed[si]); si += 1
        else:
            order.append(cached[ic]); ic += 1
    assert sorted(order) == list(range(n_total))
    return order


def _build_and_compile():
    import concourse.mybir as mybir
    import concourse.tile as tile
    from concourse import bacc
    from concourse.masks import make_identity

    dt = mybir.dt
    f32 = dt.float32
    bf16 = dt.bfloat16
    AF = mybir.ActivationFunctionType

    nc = bacc.Bacc("TRN2", target_bir_lowering=False, debug=False,
                   num_devices=RANKS)

    adjTn = nc.dram_tensor("adjTn", [NK, ML], bf16, kind="ExternalInput").ap()
    adjTr = nc.dram_tensor("adjTr", [NK, ML], bf16, kind="ExternalInput").ap()
    xT = nc.dram_tensor("xT", [F, NK], bf16, kind="ExternalInput").ap()
    w1 = nc.dram_tensor("w1", [F, HID], bf16, kind="ExternalInput").ap()
    w2 = nc.dram_tensor("w2", [HID, C], bf16, kind="ExternalInput").ap()
    hm = nc.dram_tensor("hm", [C, C], bf16, kind="ExternalInput").ap()
    b1 = nc.dram_tensor("b1", [HID, 1], f32, kind="ExternalInput").ap()
    b2c = nc.dram_tensor("b2c", [C, 1], f32, kind="ExternalInput").ap()
    outT = nc.dram_tensor("outT", [C, ML], f32, kind="ExternalOutput").ap()

    rg = [list(range(RANKS))]

    with tile.TileContext(nc) as tc:
        with tc.tile_pool(name="const", bufs=1) as const_pool, \
             tc.tile_pool(name="persist", bufs=1) as persist, \
             tc.tile_pool(name="dram", bufs=1, space="DRAM") as dram_pool:

            # ---- constants ----
            w1_sb = const_pool.tile([P, FT, HID], bf16)
            nc.sync.dma_start(w1_sb[:], w1.rearrange("(kt p) h -> p kt h", p=P))
            w2_sb = const_pool.tile([P, 2, C], bf16)
            nc.sync.dma_start(w2_sb[:], w2.rearrange("(kt p) c -> p kt c", p=P))
            h_sb = const_pool.tile([C, C], bf16)
            nc.sync.dma_start(h_sb[:], hm[:])
            b1_sb = const_pool.tile([P, 2, 1], f32)
            nc.sync.dma_start(b1_sb[:], b1.rearrange("(t p) o -> p t o", p=P))
            b2c_sb = const_pool.tile([C, 1], f32)
            nc.sync.dma_start(b2c_sb[:], b2c[:])
            ones16_sb = const_pool.tile([C, 1], f32)
            nc.gpsimd.memset(ones16_sb[:], 1.0)
            ones1_sb = const_pool.tile([1, C], f32)
            nc.gpsimd.memset(ones1_sb[:], 1.0)

            # ---- persistent intermediates ----
            h1t_sb = persist.tile([P, 2, ML], bf16)        # h.T  [HID, ML]
            hw2f_sb = persist.tile([P, KT, C], bf16)       # gathered h@W2 [NK, C]
            et_sb = persist.tile([C, ML], f32)             # E_hat.T local
            btcat_sb = persist.tile([C, NK], bf16)         # gathered B.T
            y_sb = persist.tile([P, KT, C], bf16)          # (B @ H) K-major

            # ================= phase 1: XW1 = X @ W1  [NK, HID] =============
            if PHASES >= 1:
              with tc.tile_pool(name="xw1p", bufs=1) as xw1p:
                xw1_sb = xw1p.tile([P, KT, HID], bf16)
                with tc.tile_pool(name="ph1", bufs=1) as ph1, \
                     tc.tile_pool(name="ps1", bufs=4, space="PSUM") as ps1:
                    xT_sb = ph1.tile([P, FT, NK], bf16)
                    xT_r = xT.rearrange("(kt p) n -> p kt n", p=P)
                    XCH = 8
                    xw = NK // XCH
                    for c in range(XCH):
                        nc.sync.dma_start(xT_sb[:, :, c * xw:(c + 1) * xw],
                                          xT_r[:, :, c * xw:(c + 1) * xw])
                    for m in range(KT):
                        psum1 = ps1.tile([P, HID], f32, name="psum1")
                        for kf in range(FT):
                            nc.tensor.matmul(
                                psum1[:],
                                xT_sb[:, kf, m * P:(m + 1) * P],
                                w1_sb[:, kf, :],
                                start=(kf == 0), stop=(kf == FT - 1))
                        nc.scalar.activation(xw1_sb[:, m, :], psum1[:], AF.Copy)

                # ============= phase 2: H1T = relu(XW1.T @ adjTn + b1) ======
                if PHASES >= 2:
                    # cachen outlives phase 2 (reused in phase 4)
                    cachen_cm = tc.tile_pool(name="cachen", bufs=1)
                    cachen = cachen_cm.__enter__()
                    adjn_res = cachen.tile([P, NRESN, ML], bf16)
                    with tc.tile_pool(name="ph2s", bufs=4) as ph2s, \
                         tc.tile_pool(name="ps2", bufs=1, space="PSUM") as ps2:
                        psum_h0 = ps2.tile([P, ML], f32, name="psum_h0")
                        psum_h1 = ps2.tile([P, ML], f32, name="psum_h1")
                        psum_h = [psum_h0, psum_h1]
                        for k in range(KT):
                            if k < NRESN:
                                nc.sync.dma_start(adjn_res[:, k, :],
                                                  adjTn[k * P:(k + 1) * P, :])
                                src = adjn_res[:, k, :]
                            else:
                                adjn_k = ph2s.tile([P, ML], bf16, name="adjn_k")
                                nc.sync.dma_start(adjn_k[:],
                                                  adjTn[k * P:(k + 1) * P, :])
                                src = adjn_k[:]
                            for mh in range(2):
                                for (n0, n1) in CHUNKS:
                                    nc.tensor.matmul(
                                        psum_h[mh][:, n0:n1],
                                        xw1_sb[:, k, mh * P:(mh + 1) * P],
                                        src[:, n0:n1],
                                        start=(k == 0), stop=(k == KT - 1))
                        for mh in range(2):
                            nc.scalar.activation(h1t_sb[:, mh, :], psum_h[mh][:],
                                                 AF.Relu, bias=b1_sb[:, mh, :])

                # ================= phase 3: hW2 = h @ W2  [ML, C], all-gather ===
                if PHASES >= 3:
                    with tc.tile_pool(name="ph3", bufs=1) as ph3, \
                         tc.tile_pool(name="ps3", bufs=4, space="PSUM") as ps3:
                        hw2_sb = ph3.tile([P, MT, C], bf16)
                        for m in range(MT):
                            psum3 = ps3.tile([P, C], f32, name="psum3")
                            for kh in range(2):
                                nc.tensor.matmul(
                                    psum3[:],
                                    h1t_sb[:, kh, m * P:(m + 1) * P],
                                    w2_sb[:, kh, :],
                                    start=(kh == 0), stop=(kh == 1))
                            nc.scalar.activation(hw2_sb[:, m, :], psum3[:], AF.Copy)
                        hw2loc_dram = dram_pool.tile([ML, C], bf16)
                        nc.sync.dma_start(
                            hw2loc_dram.rearrange("(mt p) c -> p mt c", p=P),
                            hw2_sb[:])
                        hw2full_dram = dram_pool.tile([NK, C], bf16,
                                                      addr_space="Shared")
                        nc.gpsimd.collective_compute(
                            "AllGather", mybir.AluOpType.bypass, replica_groups=rg,
                            ins=[hw2loc_dram[:].opt()], outs=[hw2full_dram[:].opt()])
                        nc.sync.dma_start(
                            hw2f_sb[:],
                            hw2full_dram.rearrange("(kt p) c -> p kt c", p=P))

                # ====== phase 4: logitsT = hW2_full.T @ adjTn; softmax; E_hat ===
                if PHASES >= 4:
                    with tc.tile_pool(name="ph4s", bufs=4) as ph4s, \
                         tc.tile_pool(name="ph4", bufs=1) as ph4, \
                         tc.tile_pool(name="ps4", bufs=1, space="PSUM") as ps4:
                        psum_l0 = ps4.tile([P, ML], f32, name="psum_l0",
                                           tag="ph4big")
                        psum_l1 = ps4.tile([P, ML], f32, name="psum_l1")
                        psum_ls = [psum_l0, psum_l1]
                        korder = _mix_order(NRESN, KT)
                        for ki, k in enumerate(korder):
                            j = ki % 2  # PE column strip
                            if k < NRESN:
                                src = adjn_res[:, k, :]
                            else:
                                adjn_k2 = ph4s.tile([P, ML], bf16, name="adjn_k2")
                                nc.sync.dma_start(adjn_k2[:],
                                                  adjTn[k * P:(k + 1) * P, :])
                                src = adjn_k2[:]
                            for (n0, n1) in CHUNKS:
                                nc.tensor.matmul(
                                    psum_ls[j][32 * j:32 * j + C, n0:n1],
                                    hw2f_sb[:, k, :],
                                    src[:, n0:n1],
                                    start=(ki < 2), stop=(ki >= KT - 2),
                                    tile_position=(0, 32 * j))
                        # sum the 2 column-strip partials -> logitsT [C, ML]
                        # (DVE may read only one PSUM operand: stage strip 1 via SBUF)
                        lt_s1 = ph4.tile([C, ML], f32, name="lt_s1")
                        nc.scalar.activation(lt_s1[:], psum_l1[32:32 + C, :], AF.Copy)
                        lt_sum = ph4.tile([C, ML], f32, name="lt_sum")
                        nc.vector.tensor_add(lt_sum[:], psum_l0[0:C, :], lt_s1[:])
                        # transposed softmax over classes (partition dim):
                        # expT = exp(logitsT + b2); sums = 1^T expT (PE);
                        # bcast sums over partitions (PE); E = expT/sums - 1/C
                        expT_sb = ph4.tile([C, ML], f32)
                        nc.scalar.activation(expT_sb[:], lt_sum[:], AF.Exp,
                                             bias=b2c_sb[:])
                        sums_ps = ps4.tile([1, ML], f32, name="sums_ps",
                                           tag="ph4big")
                        for (n0, n1) in CHUNKS:
                            nc.tensor.matmul(sums_ps[:, n0:n1], ones16_sb[:],
                                             expT_sb[:, n0:n1],
                                             start=True, stop=True)
                        sumsr_sb = ph4.tile([1, ML], f32)
                        nc.scalar.activation(sumsr_sb[:], sums_ps[:], AF.Copy)
                        bc_ps = ps4.tile([C, ML], f32, name="bc_ps", tag="ph4big")
                        for (n0, n1) in CHUNKS:
                            nc.tensor.matmul(bc_ps[:, n0:n1], ones1_sb[:],
                                             sumsr_sb[:, n0:n1],
                                             start=True, stop=True)
                        rcp_sb = ph4.tile([C, ML], f32)
                        nc.vector.reciprocal(rcp_sb[:], bc_ps[:])
                        et_pre = ph4.tile([C, ML], f32)
                        nc.vector.tensor_mul(et_pre[:], expT_sb[:], rcp_sb[:])
                        nc.vector.tensor_scalar_add(et_sb[:], et_pre[:], -1.0 / C)
                        etb_sb = ph4.tile([C, ML], bf16)
                        nc.scalar.activation(etb_sb[:], et_sb[:], AF.Copy)

                        # all-gather E_hat.T blocks -> btcat
                        et_dram = dram_pool.tile([C, ML], bf16)
                        nc.sync.dma_start(et_dram[:], etb_sb[:])
                        btfull0 = dram_pool.tile([P, ML], bf16, addr_space="Shared")
                        nc.gpsimd.collective_compute(
                            "AllGather", mybir.AluOpType.bypass, replica_groups=rg,
                            ins=[et_dram[:].opt()], outs=[btfull0[:].opt()])
                        nc.sync.dma_start(
                            btcat_sb.rearrange("c (r m) -> c r m", r=RANKS),
                            btfull0.rearrange("(r c) m -> c r m", c=C))

                if PHASES >= 2:
                    cachen_cm.__exit__(None, None, None)

            # ================= phase 5: post-process iterations =============
            if PHASES >= 5:
                with tc.tile_pool(name="res", bufs=1) as res_pool, \
                     tc.tile_pool(name="ph5s", bufs=4) as ph5s, \
                     tc.tile_pool(name="ph5", bufs=1) as ph5, \
                     tc.tile_pool(name="ps5y", bufs=2, space="PSUM") as ps5y, \
                     tc.tile_pool(name="ps5b", bufs=1, space="PSUM") as ps5b:
                    adjr_res = res_pool.tile([P, NRES, ML], bf16)
                    for it in range(NPOST):
                        # Y = B @ H in K-major layout, from gathered B.T blocks
                        YB = 32
                        for mb in range(0, KT, YB):
                            nb = min(YB, KT - mb)
                            psum_y = ps5y.tile([P, YB, C], f32, name="psum_y")
                            for j in range(nb):
                                m = mb + j
                                nc.tensor.matmul(psum_y[:, j, :],
                                                 btcat_sb[:, m * P:(m + 1) * P],
                                                 h_sb[:], start=True, stop=True)
                            nc.scalar.activation(y_sb[:, mb:mb + nb, :],
                                                 psum_y[:, :nb, :], AF.Copy)
                        # T.T = Y.T @ adjTr  (accumulate over k-tiles)
                        psum_b0 = ps5b.tile([P, ML], f32, name="psum_b0")
                        psum_b1 = ps5b.tile([P, ML], f32, name="psum_b1")
                        psum_bs = [psum_b0, psum_b1]
                        korder5 = _mix_order(NRES, KT) if it > 0 else list(range(KT))
                        for ki, k in enumerate(korder5):
                            j = ki % 2  # PE column strip
                            if k < NRES:
                                if it == 0:
                                    nc.sync.dma_start(
                                        adjr_res[:, k, :],
                                        adjTr[k * P:(k + 1) * P, :])
                                src = adjr_res[:, k, :]
                            else:
                                adjr_k = ph5s.tile([P, ML], bf16, name="adjr_k")
                                nc.sync.dma_start(adjr_k[:],
                                                  adjTr[k * P:(k + 1) * P, :])
                                src = adjr_k[:]
                            for (n0, n1) in CHUNKS:
                                nc.tensor.matmul(
                                    psum_bs[j][32 * j:32 * j + C, n0:n1],
                                    y_sb[:, k, :],
                                    src[:, n0:n1],
                                    start=(ki < 2), stop=(ki >= KT - 2),
                                    tile_position=(0, 32 * j))
                        bt_s1 = ph5.tile([C, ML], f32, name="bt_s1", bufs=2)
                        nc.scalar.activation(bt_s1[:], psum_b1[32:32 + C, :], AF.Copy)
                        btsum = ph5.tile([C, ML], f32, name="btsum", bufs=2)
                        nc.vector.tensor_add(btsum[:], psum_b0[0:C, :], bt_s1[:])
                        if it < NPOST - 1:
                            btnb = ph5.tile([C, ML], bf16, name="btnb", bufs=2)
                            nc.vector.tensor_add(btnb[:], btsum[:], et_sb[:])
                            bt_dram = dram_pool.tile([C, ML], bf16,
                                                     name=f"bt_dram{it}")
                            nc.sync.dma_start(bt_dram[:], btnb[:])
                            btfull = dram_pool.tile([P, ML], bf16,
                                                    name=f"btfull{it}",
                                                    addr_space="Shared")
                            nc.gpsimd.collective_compute(
                                "AllGather", mybir.AluOpType.bypass,
                                replica_groups=rg,
                                ins=[bt_dram[:].opt()], outs=[btfull[:].opt()])
                            nc.sync.dma_start(
                                btcat_sb.rearrange("c (r m) -> c r m", r=RANKS),
                                btfull.rearrange("(r c) m -> c r m", c=C))
                        else:
                            btn = ph5.tile([C, ML], f32, name="btn")
                            nc.vector.tensor_add(btn[:], btsum[:], et_sb[:])
                            outT_sb = ph5.tile([C, ML], f32, name="outT_sb")
                            nc.vector.tensor_scalar_add(outT_sb[:], btn[:],
                                                        1.0 / C)
                            nc.sync.dma_start(outT[:], outT_sb[:])
            else:
                # truncated build: still write the output tensor
                with tc.tile_pool(name="dummy", bufs=1) as dummy:
                    dpad = dummy.tile([C, ML], f32)
                    nc.gpsimd.memset(dpad[:], 0.0)
                    nc.sync.dma_start(outT[:], dpad[:])

    nc.compile()
    return nc


def _get_compiled():
    if "nc" not in _CACHE:
        _CACHE["nc"] = _build_and_compile()
    return _CACHE["nc"]


def _prep_inputs(raw_adj, normed_adj, features, W1, b1, W2, b2, H):
    bf = ml_dtypes.bfloat16
    xTp = np.zeros((F, NK), dtype=bf)
    xTp[:, :NREAL] = np.ascontiguousarray(features.T).astype(bf)
    w1b = np.ascontiguousarray(W1).astype(bf)
    w2b = np.ascontiguousarray(W2).astype(bf)
    hb = np.ascontiguousarray(H).astype(bf)
    b1c = np.asarray(b1, dtype=np.float32).reshape(HID, 1).copy()
    b2col = np.asarray(b2, dtype=np.float32).reshape(C, 1).copy()
    in_maps = []
    for r in range(RANKS):
        r0 = r * ML
        r1 = min(r0 + ML, NREAL)
        nr = r1 - r0
        an = np.zeros((NK, ML), dtype=bf)
        an[:NREAL, :nr] = np.ascontiguousarray(normed_adj[r0:r1].T).astype(bf)
        ar = np.zeros((NK, ML), dtype=bf)
        ar[:NREAL, :nr] = np.ascontiguousarray(raw_adj[r0:r1].T).astype(bf)
        in_maps.append({
            "adjTn": an, "adjTr": ar, "xT": xTp, "w1": w1b, "w2": w2b,
            "hm": hb, "b1": b1c, "b2c": b2col,
        })
    return in_maps


def run_on_device(in_maps, trace=False):
    from concourse import bass_utils
    nc = _get_compiled()
    return bass_utils.run_bass_kernel_spmd(
        nc, in_maps, core_ids=list(range(RANKS)), trace=trace)


def kernel(raw_adj, normed_adj, features, y_onehot, train_mask,
           W1, b1, W2, b2, H):
    in_maps = _prep_inputs(np.asarray(raw_adj), np.asarray(normed_adj),
                           np.asarray(features), np.asarray(W1),
                           np.asarray(b1), np.asarray(W2), np.asarray(b2),
                           np.asarray(H))
    res = run_on_device(in_maps)
    parts = []
    for r in range(RANKS):
        o = np.asarray(res.results[r]["outT"], dtype=np.float32)  # [C, ML]
        parts.append(o.T)
    full = np.concatenate(parts, axis=0)[:NREAL]
    return np.ascontiguousarray(full).astype(np.float32)



# revision 18
# speedup vs baseline: 1.3425x; 1.3425x over previous
"""CPGNN (compatibility-guided GNN) kernel for 8 Trainium2 NeuronCores.

Reference computation (N=10000, F=512, HID=256, C=16, 4 post iterations):
    h      = relu(normed_adj @ (features @ W1) + b1)
    logits = normed_adj @ (h @ W2) + b2
    E_hat  = softmax(logits) - 1/C
    B_hat  = E_hat;  4x: B_hat = E_hat + raw_adj @ (B_hat @ H)
    out    = B_hat + 1/C

Sharding: rows of both adjacency matrices are sharded over the 8 cores
(1280 rows per core, tail core padded).  Adjacency shards are uploaded
TRANSPOSED and PAIR-MAJOR in fp8-e4m3 (scaled by 2^20 / 2^15), so on-
device matmuls stream half the bytes of bf16 and phase 4 can use the
fp8 DoubleRow perf mode (2x PE throughput).  The small per-node
matrices (XW1, h, B, Y) stay bf16/f32 — fp8 there fails the error
budget (verified by simulation).  Mixed-precision matmuls (bf16
stationary x fp8 moving) are used for phases 2 and 5.

Pipeline: phase 1 (X@W1) is fused tile-by-tile into phase 2's
accumulation loop; raw_adj is prefetched into SBUF during phases 1-4
so all 4 post iterations run from SBUF; Y = B@H is computed locally
BEFORE each all-gather (payload [1280,16]); two tiny warm-up
all-gathers are issued at kernel start to warm the collective rings.

Measured: rel err ~4e-3 end to end.
"""

import numpy as np
import ml_dtypes

RANKS = 8
P = 128
NREAL = 10000
NK = 10240            # padded global row count
ML = 1280             # local rows per core
KT = NK // P          # 80 k-tiles
NPAIR = KT // 2       # 40 k-tile pairs
MT = ML // P          # 10
F = 512
FT = F // P           # 4
HID = 256
C = 16
NPOST = 4
NCACHE = 14           # adjn pairs cached in SBUF for phase 4 reuse
CHUNKS = [(0, 512), (512, 1024), (1024, 1280)]

# NOTE: device float8e4 is e4m3 WITH inf/NaN (max finite 240, bytes
# >= 0x78 decode as inf/nan on the PE) — keep every fp8 value <= 224.
SA_N = 2.0 ** 19      # normed_adj fp8 scale (max ~107)
SA_R = 2.0 ** 14      # raw_adj fp8 scale (max ~164)
S_HW2 = 2.0 ** 12     # h@W2 fp8 scale (max ~130)
SE = SA_R             # B_hat carried as SE * B (fp32) on device

_CACHE = {}


def _mix_order(n_cache, n_total):
    """Interleave cached (0..n_cache-1) and streamed (n_cache..) indices so
    DMA of streamed tiles overlaps PE work on cached tiles evenly."""
    cached = list(range(n_cache))
    streamed = list(range(n_cache, n_total))
    order = []
    ic = si = 0
    for i in range(n_total):
        want_stream = streamed and (si + 1) / len(streamed) <= (i + 1) / n_total
        if si < len(streamed) and (ic >= len(cached) or want_stream):
            order.append(streamed[si]); si += 1
        else:
            order.append(cached[ic]); ic += 1
    assert sorted(order) == list(range(n_total))
    return order


def _build_and_compile():
    import concourse.mybir as mybir
    import concourse.tile as tile
    from concourse import bacc

    dt = mybir.dt
    f32 = dt.float32
    bf16 = dt.bfloat16
    f8 = dt.float8e4
    AF = mybir.ActivationFunctionType
    DR = mybir.MatmulPerfMode.DoubleRow

    nc = bacc.Bacc("TRN2", target_bir_lowering=False, debug=False,
                   num_devices=RANKS)

    adjn8 = nc.dram_tensor("adjn8", [NPAIR, P, 2 * ML], f8,
                           kind="ExternalInput").ap()
    adjr8 = nc.dram_tensor("adjr8", [NPAIR, P, 2 * ML], f8,
                           kind="ExternalInput").ap()
    xk = nc.dram_tensor("xk", [KT, P, F], bf16, kind="ExternalInput").ap()
    w1 = nc.dram_tensor("w1", [F, HID], bf16, kind="ExternalInput").ap()
    w2 = nc.dram_tensor("w2", [HID, C], bf16, kind="ExternalInput").ap()
    hm = nc.dram_tensor("hm", [C, C], bf16, kind="ExternalInput").ap()
    b1 = nc.dram_tensor("b1", [HID, 1], f32, kind="ExternalInput").ap()
    b2c = nc.dram_tensor("b2c", [C, 1], f32, kind="ExternalInput").ap()
    outT = nc.dram_tensor("outT", [C, ML], f32, kind="ExternalOutput").ap()

    rg = [list(range(RANKS))]

    with tile.TileContext(nc) as tc:
        with tc.tile_pool(name="const", bufs=1) as const_pool, \
             tc.tile_pool(name="persist", bufs=1) as persist, \
             tc.tile_pool(name="res", bufs=1) as res_pool, \
             tc.tile_pool(name="dram", bufs=1, space="DRAM") as dram_pool:

            # ---- constants ----
            w1_sb = const_pool.tile([P, FT, HID], bf16)
            nc.sync.dma_start(w1_sb[:], w1.rearrange("(kt p) h -> p kt h", p=P))
            w2_sb = const_pool.tile([P, 2, C], bf16)
            nc.sync.dma_start(w2_sb[:], w2.rearrange("(kt p) c -> p kt c", p=P))
            h_sb = const_pool.tile([C, C], bf16)
            nc.sync.dma_start(h_sb[:], hm[:])
            b1_sb = const_pool.tile([P, 2, 1], f32)
            nc.sync.dma_start(b1_sb[:], b1.rearrange("(t p) o -> p t o", p=P))
            b2c_sb = const_pool.tile([C, 1], f32)
            nc.sync.dma_start(b2c_sb[:], b2c[:])
            ones16_sb = const_pool.tile([C, 1], f32)
            nc.gpsimd.memset(ones16_sb[:], 1.0)
            ones1_sb = const_pool.tile([1, C], f32)
            nc.gpsimd.memset(ones1_sb[:], 1.0)

            # ---- warm-up collectives (overlap phase 1/2 compute) ----
            wu_in = dram_pool.tile([C, 1], f32)
            nc.sync.dma_start(wu_in[:], ones16_sb[:])
            for wi in range(3):
                wu_out = dram_pool.tile([RANKS * C, 1], f32,
                                        name=f"wu_out{wi}",
                                        addr_space="Shared")
                nc.gpsimd.collective_compute(
                    "AllGather", mybir.AluOpType.bypass, replica_groups=rg,
                    ins=[wu_in[:].opt()], outs=[wu_out[:].opt()])

            # ---- persistent intermediates ----
            h1t_sb = persist.tile([P, 2, ML], bf16)      # h.T  [HID, ML]
            hw2f_sb = persist.tile([P, KT, C], f8)       # gathered h@W2 (fp8)
            y_sb = persist.tile([P, KT, C], bf16)        # gathered Y = B@H
            e15_sb = persist.tile([C, ML], f32)          # 2^15 * E_hat.T local
            yb0_sb = persist.tile([C, ML], bf16)         # E_hat.T in bf16
            adjr_res = res_pool.tile([P, NPAIR, 2, ML], f8)  # full raw shard

            # ====== fused phase 1+2: XW1 tiles -> h.T accumulation =========
            with tc.tile_pool(name="adjnc", bufs=1) as adjnc:
                adjn_res = adjnc.tile([P, NCACHE, 2, ML], f8)
                with tc.tile_pool(name="xs", bufs=6) as xs, \
                     tc.tile_pool(name="xw1p", bufs=4) as xw1p, \
                     tc.tile_pool(name="ph2s", bufs=4) as ph2s, \
                     tc.tile_pool(name="ps1", bufs=2, space="PSUM") as ps1, \
                     tc.tile_pool(name="ps2", bufs=1, space="PSUM") as ps2:
                    psum_h0 = ps2.tile([P, ML], f32, name="psum_h0")
                    psum_h1 = ps2.tile([P, ML], f32, name="psum_h1")
                    psum_h = [psum_h0, psum_h1]

                    def ph1_tile(k):
                        xt = xs.tile([P, FT, P], bf16, name="xt")
                        nc.sync.dma_start(
                            xt[:], xk[k].rearrange("p (kf j) -> p kf j", kf=FT))
                        psum1 = ps1.tile([P, HID], f32, name="psum1")
                        for kf in range(FT):
                            nc.tensor.matmul(psum1[:], xt[:, kf, :],
                                             w1_sb[:, kf, :],
                                             start=(kf == 0),
                                             stop=(kf == FT - 1))
                        xw1k = xw1p.tile([P, HID], bf16, name="xw1k")
                        nc.scalar.activation(xw1k[:], psum1[:], AF.Copy)
                        return xw1k

                    def ph2_tile(k, xw1k, src):
                        # src: [P, ML] fp8 view of adjn k-tile
                        for mh in range(2):
                            for (n0, n1) in CHUNKS:
                                nc.tensor.matmul(
                                    psum_h[mh][:, n0:n1],
                                    xw1k[:, mh * P:(mh + 1) * P],
                                    src[:, n0:n1],
                                    start=(k == 0), stop=(k == KT - 1))

                    prev = None  # (k, xw1k, src)
                    for k in range(KT):
                        pr, half = divmod(k, 2)
                        if half == 0:
                            if pr < NCACHE:
                                pair_t = adjn_res[:, pr, :, :]
                            else:
                                pair_t = ph2s.tile([P, 2, ML], f8,
                                                   name="adjn_k")
                            nc.sync.dma_start(
                                pair_t[:],
                                adjn8[pr].rearrange("p (two m) -> p two m",
                                                    two=2))
                            # interleave raw-adj prefetch 1:1 with adjn stream
                            nc.sync.dma_start(
                                adjr_res[:, pr, :, :],
                                adjr8[pr].rearrange("p (two m) -> p two m",
                                                    two=2))
                            cur_pair = pair_t
                        xw1k = ph1_tile(k)
                        if prev is not None:
                            ph2_tile(*prev)
                        prev = (k, xw1k, cur_pair[:, half, :])
                    ph2_tile(*prev)
                    for mh in range(2):
                        nc.scalar.activation(h1t_sb[:, mh, :], psum_h[mh][:],
                                             AF.Relu, bias=b1_sb[:, mh, :],
                                             scale=1.0 / SA_N)

                # ====== phase 3: hW2 = h @ W2 -> fp8, all-gather ============
                with tc.tile_pool(name="ph3", bufs=1) as ph3, \
                     tc.tile_pool(name="ps3", bufs=4, space="PSUM") as ps3:
                    hw2_sb = ph3.tile([P, MT, C], f8)
                    for m in range(MT):
                        psum3 = ps3.tile([P, C], f32, name="psum3")
                        for kh in range(2):
                            nc.tensor.matmul(
                                psum3[:],
                                h1t_sb[:, kh, m * P:(m + 1) * P],
                                w2_sb[:, kh, :],
                                start=(kh == 0), stop=(kh == 1))
                        nc.scalar.activation(hw2_sb[:, m, :], psum3[:],
                                             AF.Copy, scale=S_HW2)
                    hw2loc_dram = dram_pool.tile([ML, C], f8)
                    nc.sync.dma_start(
                        hw2loc_dram.rearrange("(mt p) c -> p mt c", p=P),
                        hw2_sb[:])
                    hw2full_dram = dram_pool.tile([NK, C], f8,
                                                  addr_space="Shared")
                    nc.gpsimd.collective_compute(
                        "AllGather", mybir.AluOpType.bypass, replica_groups=rg,
                        ins=[hw2loc_dram[:].opt()],
                        outs=[hw2full_dram[:].opt()])
                    nc.sync.dma_start(
                        hw2f_sb[:],
                        hw2full_dram.rearrange("(kt p) c -> p kt c", p=P))

                # ====== phase 4: logits via fp8 DoubleRow; softmax; E =======
                with tc.tile_pool(name="ph4s", bufs=4) as ph4s, \
                     tc.tile_pool(name="ph4", bufs=1) as ph4, \
                     tc.tile_pool(name="ps4", bufs=1, space="PSUM") as ps4:
                    psum_l0 = ps4.tile([P, ML], f32, name="psum_l0",
                                       tag="ph4big")
                    order = _mix_order(NCACHE, NPAIR)
                    for pi, pr in enumerate(order):
                        if pr < NCACHE:
                            src = adjn_res[:, pr, :, :]
                        else:
                            src = ph4s.tile([P, 2, ML], f8, name="adjn_k2")
                            nc.sync.dma_start(
                                src[:],
                                adjn8[pr].rearrange("p (two m) -> p two m",
                                                    two=2))
                        for (n0, n1) in CHUNKS:
                            nc.tensor.matmul(
                                psum_l0[0:C, n0:n1],
                                hw2f_sb[:, 2 * pr:2 * pr + 2, :],
                                src[:, :, n0:n1],
                                start=(pi == 0), stop=(pi == NPAIR - 1),
                                perf_mode=DR)
                    # transposed softmax: expT = exp(logits + b2)
                    # (psum_l0 holds 2^33 * logits.T)
                    expT_sb = ph4.tile([C, ML], f32)
                    nc.scalar.activation(expT_sb[:], psum_l0[0:C, :], AF.Exp,
                                         bias=b2c_sb[:],
                                         scale=1.0 / (SA_N * S_HW2))
                    sums_ps = ps4.tile([1, ML], f32, name="sums_ps",
                                       tag="soft")
                    for (n0, n1) in CHUNKS:
                        nc.tensor.matmul(sums_ps[:, n0:n1], ones16_sb[:],
                                         expT_sb[:, n0:n1],
                                         start=True, stop=True)
                    sumsr_sb = ph4.tile([1, ML], f32)
                    nc.scalar.activation(sumsr_sb[:], sums_ps[:], AF.Copy)
                    bc_ps = ps4.tile([C, ML], f32, name="bc_ps",
                                     tag="soft")
                    for (n0, n1) in CHUNKS:
                        nc.tensor.matmul(bc_ps[:, n0:n1], ones1_sb[:],
                                         sumsr_sb[:, n0:n1],
                                         start=True, stop=True)
                    rcp_sb = ph4.tile([C, ML], f32)
                    nc.vector.reciprocal_approx_fast(rcp_sb[:], bc_ps[:])
                    prior_sb = ph4.tile([C, ML], f32)
                    nc.vector.tensor_mul(prior_sb[:], expT_sb[:], rcp_sb[:])
                    # E15 = 2^15*(prior - 1/C); yb0 = bf16(prior - 1/C)
                    nc.scalar.activation(e15_sb[:], prior_sb[:], AF.Copy,
                                         scale=SE, bias=-SE / C)
                    nc.scalar.activation(yb0_sb[:], prior_sb[:], AF.Copy,
                                         bias=-1.0 / C)

            # ====== phase 5: post-process iterations ========================
            with tc.tile_pool(name="ph5", bufs=2) as ph5, \
                 tc.tile_pool(name="ps5y", bufs=2, space="PSUM") as ps5y, \
                 tc.tile_pool(name="ps5b", bufs=1, space="PSUM") as ps5b:

                def y_gather(yb, it):
                    """local Y = B@H (K-major), all-gather into y_sb."""
                    psum_y = ps5y.tile([P, MT, C], f32, name="psum_y")
                    for m in range(MT):
                        nc.tensor.matmul(psum_y[:, m, :],
                                         yb[:, m * P:(m + 1) * P],
                                         h_sb[:], start=True, stop=True)
                    yloc_sb = ph5.tile([P, MT, C], bf16, name="yloc")
                    nc.scalar.activation(yloc_sb[:], psum_y[:], AF.Copy)
                    yloc_dram = dram_pool.tile([ML, C], bf16,
                                               name=f"yloc_dram{it}")
                    nc.sync.dma_start(
                        yloc_dram.rearrange("(mt p) c -> p mt c", p=P),
                        yloc_sb[:])
                    yfull = dram_pool.tile([NK, C], bf16, name=f"yfull{it}",
                                           addr_space="Shared")
                    nc.gpsimd.collective_compute(
                        "AllGather", mybir.AluOpType.bypass, replica_groups=rg,
                        ins=[yloc_dram[:].opt()], outs=[yfull[:].opt()])
                    nc.sync.dma_start(
                        y_sb[:], yfull.rearrange("(kt p) c -> p kt c", p=P))

                y_gather(yb0_sb, 0)
                for it in range(NPOST):
                    psum_b0 = ps5b.tile([P, ML], f32, name="psum_b0")
                    psum_b1 = ps5b.tile([P, ML], f32, name="psum_b1")
                    psum_bs = [psum_b0, psum_b1]
                    for k in range(KT):
                        j = k % 2
                        pr, half = divmod(k, 2)
                        src = adjr_res[:, pr, half, :]
                        for (n0, n1) in CHUNKS:
                            nc.tensor.matmul(
                                psum_bs[j][32 * j:32 * j + C, n0:n1],
                                y_sb[:, k, :],
                                src[:, n0:n1],
                                start=(k < 2), stop=(k >= KT - 2),
                                tile_position=(0, 32 * j))
                    # btS = 2^15*B = strips + E15   (psum is 2^15 * R@Y)
                    bt_s1 = ph5.tile([C, ML], f32, name="bt_s1")
                    nc.scalar.activation(bt_s1[:], psum_b1[32:32 + C, :],
                                         AF.Copy)
                    t_add = ph5.tile([C, ML], f32, name="t_add")
                    nc.vector.tensor_add(t_add[:], psum_b0[0:C, :], bt_s1[:])
                    btS = ph5.tile([C, ML], f32, name="btS")
                    nc.vector.tensor_add(btS[:], t_add[:], e15_sb[:])
                    if it < NPOST - 1:
                        yb_it = ph5.tile([C, ML], bf16, name="yb_it")
                        nc.scalar.activation(yb_it[:], btS[:], AF.Copy,
                                             scale=1.0 / SE)
                        y_gather(yb_it, it + 1)
                    else:
                        outT_sb = ph5.tile([C, ML], f32, name="outT_sb")
                        nc.scalar.activation(outT_sb[:], btS[:], AF.Copy,
                                             scale=1.0 / SE, bias=1.0 / C)
                        nc.sync.dma_start(outT[:], outT_sb[:])

    nc.compile()
    return nc


def _get_compiled():
    if "nc" not in _CACHE:
        _CACHE["nc"] = _build_and_compile()
    return _CACHE["nc"]


def _pair_major_fp8(adj_shard_T, scale):
    """[NK, ML] f32 (transposed shard) -> pair-major fp8 [NPAIR, P, 2*ML]."""
    e4 = ml_dtypes.float8_e4m3fn
    a = np.clip(adj_shard_T * np.float32(scale), 0.0, 224.0)
    a = a.reshape(NPAIR, 2, P, ML).transpose(0, 2, 1, 3).reshape(
        NPAIR, P, 2 * ML)
    return np.ascontiguousarray(a).astype(e4)


def _prep_inputs(raw_adj, normed_adj, features, W1, b1, W2, b2, H):
    bf = ml_dtypes.bfloat16
    xpad = np.zeros((NK, F), dtype=np.float32)
    xpad[:NREAL] = features
    # xk[k, p, (kf j)] = X[k*128+j, kf*128+p]
    xkarr = np.ascontiguousarray(
        xpad.reshape(KT, P, FT, P).transpose(0, 3, 2, 1).reshape(KT, P, F)
    ).astype(bf)
    w1b = np.ascontiguousarray(W1).astype(bf)
    w2b = np.ascontiguousarray(W2).astype(bf)
    hb = np.ascontiguousarray(H).astype(bf)
    b1c = np.asarray(b1, dtype=np.float32).reshape(HID, 1).copy()
    b2col = np.asarray(b2, dtype=np.float32).reshape(C, 1).copy()
    in_maps = []
    for r in range(RANKS):
        r0 = r * ML
        r1 = min(r0 + ML, NREAL)
        nr = r1 - r0
        an = np.zeros((NK, ML), dtype=np.float32)
        an[:NREAL, :nr] = normed_adj[r0:r1].T
        ar = np.zeros((NK, ML), dtype=np.float32)
        ar[:NREAL, :nr] = raw_adj[r0:r1].T
        in_maps.append({
            "adjn8": _pair_major_fp8(an, SA_N),
            "adjr8": _pair_major_fp8(ar, SA_R),
            "xk": xkarr, "w1": w1b, "w2": w2b,
            "hm": hb, "b1": b1c, "b2c": b2col,
        })
    return in_maps


def run_on_device(in_maps, trace=False):
    from concourse import bass_utils
    nc = _get_compiled()
    return bass_utils.run_bass_kernel_spmd(
        nc, in_maps, core_ids=list(range(RANKS)), trace=trace)


def kernel(raw_adj, normed_adj, features, y_onehot, train_mask,
           W1, b1, W2, b2, H):
    in_maps = _prep_inputs(np.asarray(raw_adj), np.asarray(normed_adj),
                           np.asarray(features), np.asarray(W1),
                           np.asarray(b1), np.asarray(W2), np.asarray(b2),
                           np.asarray(H))
    res = run_on_device(in_maps)
    parts = []
    for r in range(RANKS):
        o = np.asarray(res.results[r]["outT"], dtype=np.float32)  # [C, ML]
        parts.append(o.T)
    full = np.concatenate(parts, axis=0)[:NREAL]
    return np.ascontiguousarray(full).astype(np.float32)
